# revision 13
# baseline (speedup 1.0000x reference)
"""Trainium2 Bass kernel v3 for MaterialsGraphSAGE.

Changes vs v2 (1713us):
- Host-side node permutation: nodes assigned to (core, block) bins with a
  2D-balanced greedy (lo/hi in-edge counts vs 768 caps) + rank alignment of
  bins across cores -> fewer padded gather tiles (Q7 desc-gen is the
  bottleneck at ~3.4ns/idx, so padded idx cost real time).
- One-hot scatter matrices are pure 0/1 in FP8 (exact) -> half the DMA bytes;
  1/deg is applied in the PSUM->SBUF copy (tensor_tensor mult with a
  broadcast inv-deg row) instead of being folded into the one-hot.
- AllGather of the layer table is split into two chunks so the first chunk
  overlaps the tail of the layer's compute.
- dynamic_dma_scratch_size=32768 (SWDGE ring carveout 2048 descs/queue) to
  let the Q7 run further ahead of the gather DMAs.
"""

import sys

for _p in ("/opt/trn_rl_repo",):
    if _p not in sys.path:
        sys.path.insert(0, _p)

import numpy as np
import ml_dtypes

import concourse.bacc as bacc
import concourse.mybir as mybir
import concourse.tile as tile
from concourse import bass
from concourse.bass_utils import run_bass_kernel_spmd
from concourse.vector_clock import ScopedClock

F32 = mybir.dt.float32
BF16 = mybir.dt.bfloat16
F8E4 = mybir.dt.float8e4
I32 = mybir.dt.int32
I16 = mybir.dt.int16
BF = ml_dtypes.bfloat16
F8 = ml_dtypes.float8_e4m3

P = 128
NCORES = 8
NN = 50000
NG = 256
SLICE = 6272
PADN = SLICE * NCORES      # 50176
NB = SLICE // P            # 49
NL = 4
H = 128
NODE_F = 64
CH = 16                    # gather tiles per call
NEG_SLOPE = 0.01
BN_EPS = 1e-5
CC_SPLIT = 25              # blocks in the first AllGather chunk


# ---------------------------------------------------------------------------
def _patch_tile_drain():
    def _drain_and_barrier(self, tick_clock, wait_clock):
        drain_inst = self.nc.sync.drain()
        wait_clock.add_sem_waits(
            drain_inst.ins, ScopedClock({None: tick_clock.global_clock})
        )
        si = drain_inst.ins.sync_info
        waits = list(si.on_wait) if si is not None else []
        if len(waits) > 1:
            drain_inst.ins.sync_info = mybir.SyncInfo(
                on_wait=[waits[0]], on_update=list(si.on_update)
            )
            for w in waits[1:]:
                extra = self.nc.sync.drain()
                extra.ins.sync_info = mybir.SyncInfo(on_wait=[w], on_update=[])
        self.nc.all_engine_barrier()
        assert self.sems is not None
        popped = self.nc._tile_sem_poison_stack.pop()
        assert popped is self._sem_poison
        self.nc.clear_and_free_semaphores(list(self.sems.allocated().values()))
        self.nc.all_engine_barrier()

    tile.TileContext._drain_and_barrier = _drain_and_barrier


_patch_tile_drain()


def _legalize_sync_waits(nc, max_waits=1):
    for fn in nc.m.functions:
        for bb in fn.blocks:
            out = []
            changed = False
            for ins in bb.instructions:
                si = ins.sync_info
                if si is not None and len(si.on_wait) > max_waits:
                    waits = list(si.on_wait)
                    for w in waits[:-max_waits]:
                        nop = mybir.InstNoOp(
                            name=f"WSPLIT-{nc.next_id()}", ins=[], outs=[]
                        )
                        nop.engine = ins.engine
                        nop.sync_info = mybir.SyncInfo(on_wait=[w], on_update=[])
                        out.append(nop)
                    ins.sync_info = mybir.SyncInfo(
                        on_wait=waits[-max_waits:], on_update=list(si.on_update)
                    )
                    changed = True
                out.append(ins)
            if changed:
                bb.instructions = out


def _lrelu_np(x):
    return np.where(x >= 0, x, np.float32(NEG_SLOPE) * x).astype(np.float32)


def _build_perm(src, dst):
    """Assign each node a position (core, block, slot) so that per-core totals
    and per-(core, block) lo/hi in-edge counts are balanced.  Returns pos[n]."""
    deg = np.bincount(dst, minlength=NN)
    # stage 1: nodes -> cores, LPT on in-degree, node-count cap per core
    order = np.argsort(-deg, kind="stable")
    core_of = np.full(NN, -1, np.int64)
    load = np.zeros(NCORES, np.int64)
    cnt = np.zeros(NCORES, np.int64)
    big = np.int64(1) << 60
    for n in order:
        c = int(np.argmin(np.where(cnt < SLICE, load, big)))
        core_of[n] = c
        load[c] += deg[n]
        cnt[c] += 1
    # halves follow from core assignment (cores 0-3 are the lo half)
    is_hi = core_of[src] >= NCORES // 2
    lo_cnt = np.bincount(dst[~is_hi], minlength=NN).astype(np.int64)
    hi_cnt = np.bincount(dst[is_hi], minlength=NN).astype(np.int64)

    N_FAT = 2
    caps = np.full(NB, 6 * P, np.int64)
    caps[NB - N_FAT:] = 7 * P
    pos = np.full(NN, -1, np.int64)
    for c in range(NCORES):
        nodes = np.where(core_of == c)[0]
        w = lo_cnt[nodes] + hi_cnt[nodes]
        idx = np.argsort(-w, kind="stable")
        lo_f = np.zeros(NB, np.int64)
        hi_f = np.zeros(NB, np.int64)
        n_f = np.zeros(NB, np.int64)
        bin_nodes = [[] for _ in range(NB)]
        for ni in idx:
            n = nodes[ni]
            l, h = lo_cnt[n], hi_cnt[n]
            open_ = n_f < P
            ok = open_ & (lo_f + l <= caps) & (hi_f + h <= caps)
            if ok.any():
                # LPT: feasible bin with smallest max cap-normalized fill
                score = np.where(
                    ok,
                    np.maximum((lo_f + l) * 896 // caps,
                               (hi_f + h) * 896 // caps), big)
                b = int(np.argmin(score))
            else:
                score = np.where(open_, np.maximum(lo_f + l, hi_f + h), big)
                b = int(np.argmin(score))
            bin_nodes[b].append(n)
            lo_f[b] += l
            hi_f[b] += h
            n_f[b] += 1
        # refinement: swap nodes between bins to push every bin under the
        # (CAP, CAP) fill caps where total capacity permits.
        bin_of = np.empty(len(nodes), np.int64)
        node_ix = {int(n): k for k, n in enumerate(nodes)}
        for b in range(NB):
            for n in bin_nodes[b]:
                bin_of[node_ix[int(n)]] = b
        l_arr = lo_cnt[nodes]
        h_arr = hi_cnt[nodes]
        for _sweep in range(12):
            over = [b for b in range(NB)
                    if lo_f[b] > caps[b] or hi_f[b] > caps[b]]
            if not over:
                break
            fixed_any = False
            for bo in over:
                members = np.where(bin_of == bo)[0]
                members = members[np.argsort(-(l_arr[members]
                                               + h_arr[members]))]
                for a in members:
                    cur = (max(lo_f[bo] - caps[bo], 0)
                           + max(hi_f[bo] - caps[bo], 0))
                    if cur == 0:
                        break
                    la, ha = l_arr[a], h_arr[a]
                    new_lo = lo_f[bo] - la + l_arr
                    new_hi = hi_f[bo] - ha + h_arr
                    new_ovf = (np.maximum(new_lo - caps[bo], 0)
                               + np.maximum(new_hi - caps[bo], 0))
                    cap_p = caps[bin_of]
                    ok = (
                        (bin_of != bo)
                        & (lo_f[bin_of] + la - l_arr <= cap_p)
                        & (hi_f[bin_of] + ha - h_arr <= cap_p)
                        & (new_ovf < cur)
                    )
                    cand = np.where(ok)[0]
                    if len(cand) == 0:
                        continue
                    p_ = cand[np.argmin(new_ovf[cand])]
                    b2 = bin_of[p_]
                    dlo, dhi = la - l_arr[p_], ha - h_arr[p_]
                    lo_f[bo] -= dlo; hi_f[bo] -= dhi
                    lo_f[b2] += dlo; hi_f[b2] += dhi
                    bin_of[a], bin_of[p_] = b2, bo
                    fixed_any = True
            if not fixed_any:
                break
        bin_nodes = [[] for _ in range(NB)]
        for k, n in enumerate(nodes):
            bin_nodes[bin_of[k]].append(n)
        # rank-align bins across cores: thin bins sorted by (lo, hi), fat
        # bins pinned at the last ranks on every core.
        order_b = sorted(range(NB), key=lambda b: (caps[b], lo_f[b], hi_f[b]))
        for newb, b in enumerate(order_b):
            for slot, n in enumerate(bin_nodes[b]):
                pos[n] = c * SLICE + newb * P + slot
    assert (pos >= 0).all()
    return pos


def _prepare(src, dst, h0_pad, inv_deg):
    """Per-core edges grouped per (block, src-half) cell (int16 gather limit),
    tile counts normalized across cores. sp/oh columns are block-major; gather
    idx streams and pg are half-major (lo tiles then hi tiles)."""
    HALF = PADN // 2
    per_core = []
    cnts = np.zeros((NCORES, NB, 2), np.int64)
    for c in range(NCORES):
        base = c * SLICE
        m = (dst >= base) & (dst < base + SLICE)
        s_, d_ = src[m], dst[m]
        blk = (d_ - base) >> 7
        half = (s_ >= HALF).astype(np.int64)
        order = np.argsort(blk * 2 + half, kind="stable")
        s_, d_, blk, half = s_[order], d_[order], blk[order], half[order]
        np.add.at(cnts[c], (blk, half), 1)
        per_core.append((s_, d_ - base, blk, half))
    NT2 = np.maximum(0, -(-cnts.max(0) // P))          # [NB, 2]
    for b in range(NB):
        if NT2[b].sum() == 0:
            NT2[b, 0] = 1
    NT_lo, NT_hi = int(NT2[:, 0].sum()), int(NT2[:, 1].sum())
    NTT = NT_lo + NT_hi
    # stream positions (half-major) and block-major columns
    p_of = {}
    pos = [0, 0]
    colbase = np.zeros(NB + 1, np.int64)
    for b in range(NB):
        for h in (0, 1):
            for t in range(int(NT2[b, h])):
                p_of[(b, h, t)] = pos[h]
                pos[h] += 1
        colbase[b + 1] = colbase[b] + NT2[b].sum()

    core_data = []
    for c in range(NCORES):
        s_, doff, blk, half = per_core[c]
        key = blk * 2 + half
        bounds = np.searchsorted(key, np.arange(2 * NB + 1))
        flat = [np.zeros(NT_lo * P, np.int16), np.zeros(NT_hi * P, np.int16)]
        sp = np.zeros((P, NTT, P), F8)
        pg = [np.zeros((P, NT_lo, H), BF), np.zeros((P, NT_hi, H), BF)]
        for b in range(NB):
            col = colbase[b]
            for h in (0, 1):
                lo, hi = bounds[2 * b + h], bounds[2 * b + h + 1]
                sb = s_[lo:hi] - (HALF if h else 0)
                db = doff[lo:hi] - b * P
                n = hi - lo
                for t in range(int(NT2[b, h])):
                    seg = slice(t * P, min((t + 1) * P, n))
                    k = seg.stop - seg.start
                    ph = p_of[(b, h, t)]
                    if k > 0:
                        flat[h][ph * P:ph * P + k] = sb[seg]
                        sp[np.arange(k), col, db[seg]] = F8(1.0)
                        pg[h][:k, ph, :] = h0_pad[sb[seg]
                                                  + (HALF if h else 0)].astype(BF)
                    col += 1
        core_data.append(dict(
            idx_lo=_wrap_idx(flat[0]) if NT_lo else np.zeros((P, 8), np.int16),
            idx_hi=_wrap_idx(flat[1]) if NT_hi else np.zeros((P, 8), np.int16),
            sp=sp.reshape(P, NTT * P),
            pg_lo=pg[0].reshape(P, NT_lo * H),
            pg_hi=pg[1].reshape(P, NT_hi * H)))
    return NT2, colbase, p_of, NT_lo, NT_hi, core_data


def _wrap_idx(flat):
    n = flat.shape[0]
    assert n % 16 == 0
    buf = np.zeros((P, n // 16), np.int16)
    j = np.arange(n)
    for k in range(8):
        buf[16 * k + (j % 16), j // 16] = flat
    return buf


def kernel(x, edge_index, u, batch, W_emb, b_emb, W_l, b_l, W_r, gamma, beta,
           W_g, b_g, W_f1, b_f1, W_f2, b_f2):
    x = np.asarray(x, np.float32)
    u = np.asarray(u, np.float32)
    src0 = np.asarray(edge_index[0], np.int64)
    dst0 = np.asarray(edge_index[1], np.int64)
    batch = np.asarray(batch, np.int64)

    # ---- host precompute ------------------------------------------------
    pos = _build_perm(src0, dst0)
    src = pos[src0]
    dst = pos[dst0]

    h0 = _lrelu_np(x @ np.asarray(W_emb, np.float32)
                   + np.asarray(b_emb, np.float32))        # [NN, H]
    h0_pad = np.zeros((PADN, H), np.float32)
    h0_pad[pos] = h0
    deg = np.bincount(dst, minlength=PADN).astype(np.float32)
    inv_deg = (1.0 / np.maximum(deg, 1.0)).astype(np.float32)
    gcnt = np.bincount(batch, minlength=NG).astype(np.float32)
    ginv = (1.0 / np.maximum(gcnt, 1.0)).astype(np.float32)

    gid_full = np.full(PADN, -1.0, np.float32)
    gid_full[pos] = batch.astype(np.float32)

    NT2, colbase, p_of, NT_lo, NT_hi, core_data = _prepare(src, dst, h0_pad,
                                                           inv_deg)
    NTT = NT_lo + NT_hi
    NTB_MAX = int(NT2.sum(1).max())

    inv_std = np.float32(1.0 / np.sqrt(1.0 + BN_EPS))
    gscale = np.asarray(gamma, np.float32) * inv_std
    beta_np = np.asarray(beta, np.float32)

    giota_np = np.broadcast_to(np.arange(NG, dtype=np.float32), (P, NG)).copy()
    ident_np = np.eye(P, dtype=BF)

    gids = [gid_full[c * SLICE:(c + 1) * SLICE].reshape(NB, P).T.copy()
            for c in range(NCORES)]

    # ------------------------------------------------------------------
    nc = bacc.Bacc(None, num_swdge_queues=4, dynamic_dma_scratch_size=32768)

    def din(name, shape, dtype=F32):
        return nc.dram_tensor(name, shape, dtype, kind="ExternalInput")

    idx_lo_in = din("idx_lo", [P, max(NT_lo, 1) * 8], I16)
    idx_hi_in = din("idx_hi", [P, max(NT_hi, 1) * 8], I16)
    sp_in = din("sp", [P, NTT * P], F8E4)
    pg_lo_in = din("pg_lo", [P, max(NT_lo, 1) * H], BF16)
    pg_hi_in = din("pg_hi", [P, max(NT_hi, 1) * H], BF16)
    hT0_in = din("hT0", [P, SLICE], BF16)
    gid_in = din("gid", [P, NB])
    giota_in = din("giota", [P, NG])
    ident_in = din("ident", [P, P], BF16)
    wl_in = din("wl", [H, NL * H], BF16)
    wr_in = din("wr", [H, NL * H], BF16)
    bl_in = din("bl", [P, NL])
    gs_in = din("gs", [P, NL])
    bt_in = din("bt", [P, NL])
    ginv_in = din("ginv", [1, NG])
    invdeg_in = din("invdeg", [1, SLICE], BF16)
    uT_in = din("uT", [16, NG])
    wg_in = din("wg", [16, H])
    bg_in = din("bg", [P, 1])
    wf1_in = din("wf1", [2 * H, H])
    bf1_in = din("bf1", [P, 1])
    wf2_in = din("wf2", [H, 2])
    bf2_in = din("bf2", [2, 1])
    y_out = nc.dram_tensor("y", [2, NG], F32, kind="ExternalOutput")

    RG = [list(range(NCORES))]
    AluOp = mybir.AluOpType
    Act = mybir.ActivationFunctionType

    # interleaved lo/hi chunk schedule (shared across layers)
    HALF = PADN // 2
    chunks = {0: [], 1: []}
    for h, nt in ((0, NT_lo), (1, NT_hi)):
        for c0 in range(0, nt, CH):
            chunks[h].append((c0, min(CH, nt - c0)))
    inter = []
    i = j = 0
    while i < len(chunks[0]) or j < len(chunks[1]):
        if i < len(chunks[0]):
            inter.append((0,) + chunks[0][i]); i += 1
        if j < len(chunks[1]):
            inter.append((1,) + chunks[1][j]); j += 1

    with tile.TileContext(nc) as tc:
        with (
            tc.tile_pool(name="dram", bufs=1, space="DRAM") as dram,
            tc.tile_pool(name="meta", bufs=1) as meta,
            tc.tile_pool(name="hbuf", bufs=1) as hbuf,
            tc.tile_pool(name="glo", bufs=10) as glo,
            tc.tile_pool(name="ghi", bufs=10) as ghi,
            tc.tile_pool(name="ohp", bufs=5) as ohp,
            tc.tile_pool(name="gbp", bufs=2) as gbp,
            tc.tile_pool(name="mean", bufs=4) as meanp,
            tc.tile_pool(name="pre", bufs=4) as prep_,
            tc.tile_pool(name="stg", bufs=4) as stgp,
            tc.tile_pool(name="ps_scat", bufs=3, space="PSUM") as ps_scat,
            tc.tile_pool(name="ps_dense", bufs=2, space="PSUM") as ps_dense,
            tc.tile_pool(name="ps_tr", bufs=2, space="PSUM") as ps_tr,
            tc.tile_pool(name="ps_pool", bufs=1, space="PSUM") as ps_pool,
            tc.tile_pool(name="small", bufs=2) as small,
        ):
            idx_lo = meta.tile([P, max(NT_lo, 1) * 8], I16)
            idx_hi = meta.tile([P, max(NT_hi, 1) * 8], I16)
            gid_t = meta.tile([P, NB], F32)
            giota_t = meta.tile([P, NG], F32)
            ident_t = meta.tile([P, P], BF16)
            wl_t = meta.tile([H, NL * H], BF16)
            wr_t = meta.tile([H, NL * H], BF16)
            bl_t = meta.tile([P, NL], F32)
            gs_t = meta.tile([P, NL], F32)
            bt_t = meta.tile([P, NL], F32)
            uT_t = meta.tile([16, NG], F32)
            wg_t = meta.tile([16, H], F32)
            bg_t = meta.tile([P, 1], F32)
            wf1a_t = meta.tile([H, H], F32)
            wf1b_t = meta.tile([H, H], F32)
            bf1_t = meta.tile([P, 1], F32)
            wf2_t = meta.tile([H, 2], F32)
            bf2_t = meta.tile([2, 1], F32)
            ginv_bc = meta.tile([P, NG], F32)
            invdeg_bc = meta.tile([P, SLICE], BF16)
            for t_, i_ in (
                (idx_lo, idx_lo_in), (idx_hi, idx_hi_in),
                (gid_t, gid_in), (giota_t, giota_in),
                (ident_t, ident_in), (wl_t, wl_in), (wr_t, wr_in),
                (bl_t, bl_in), (gs_t, gs_in), (bt_t, bt_in),
                (uT_t, uT_in), (wg_t, wg_in), (bg_t, bg_in),
                (wf1a_t, wf1_in[:H, :]), (wf1b_t, wf1_in[H:, :]),
                (bf1_t, bf1_in), (wf2_t, wf2_in), (bf2_t, bf2_in),
            ):
                nc.sync.dma_start(t_[:], i_[:])
            nc.sync.dma_start(ginv_bc[:], ginv_in[:1, :].to_broadcast((P, NG)))
            nc.sync.dma_start(invdeg_bc[:],
                              invdeg_in[:1, :].to_broadcast((P, SLICE)))

            hT_a = hbuf.tile([P, SLICE], BF16, name="hT_a")
            hT_b = hbuf.tile([P, SLICE], BF16, name="hT_b")
            nc.sync.dma_start(hT_a[:], hT0_in[:])

            slices = [dram.tile([SLICE, H], BF16, name=f"slice{i}")
                      for i in range(NL - 1)]
            tables = [dram.tile([PADN, H], BF16, addr_space="Shared",
                                name=f"table{i}") for i in range(NL - 1)]
            payload_a = dram.tile([P, NG], F32, name="payload_a")
            payload_b = dram.tile([P, NG], F32, name="payload_b")
            payload_all_a = dram.tile([P, NG], F32, addr_space="Shared",
                                      name="payload_all_a")
            payload_all_b = dram.tile([P, NG], F32, addr_space="Shared",
                                      name="payload_all_b")

            hT_prev, hT_new = hT_a, hT_b
            pool_ps = ps_pool.tile([P, 2 * NG], F32, tag="pp")
            POOL_SPLIT = 25

            for li in range(NL):
                _sc = nc.enter_named_scope(f"layer{li}", False)
                chunk_tiles = {0: [], 1: []}
                qrr = 0
                for h, c0, ntc in inter:
                    g = (glo if h == 0 else ghi).tile([P, CH, H], BF16, tag="g")
                    if li == 0:
                        pgsrc = pg_lo_in if h == 0 else pg_hi_in
                        nc.sync.dma_start(
                            g[:, :ntc, :].rearrange("p a b -> p (a b)"),
                            pgsrc[:, c0 * H:(c0 + ntc) * H])
                    else:
                        idx_t_ = idx_lo if h == 0 else idx_hi
                        nidx = ntc * P
                        f0 = c0 * P // 16
                        nc.gpsimd.dma_gather(
                            out_ap=g[:, :ntc, :],
                            in_ap=tables[li - 1][h * HALF:(h + 1) * HALF, :],
                            idxs_ap=idx_t_[:, f0:f0 + nidx // 16],
                            num_idxs=nidx, num_idxs_reg=nidx, elem_size=H,
                            queue_num=qrr % 4, single_packet=False)
                        qrr += 1
                    chunk_tiles[h].append(g)

                for b in range(NB):
                    refs = ([(0, t) for t in range(int(NT2[b, 0]))]
                            + [(1, t) for t in range(int(NT2[b, 1]))])
                    ntb = len(refs)
                    col0 = int(colbase[b])
                    oh = ohp.tile([P, NTB_MAX * P], F8E4, tag="oh")
                    nc.sync.dma_start(oh[:, :ntb * P],
                                      sp_in[:, col0 * P:(col0 + ntb) * P])
                    sp = ps_scat.tile([P, P], F32, tag="sc")
                    for i_r, (h, t) in enumerate(refs):
                        ph = p_of[(b, h, t)]
                        g = chunk_tiles[h][ph // CH]
                        nc.tensor.matmul(out=sp[:],
                                         lhsT=g[:, ph % CH, :],
                                         rhs=oh[:, i_r * P:(i_r + 1) * P],
                                         start=(i_r == 0),
                                         stop=(i_r == ntb - 1))
                    mt = meanp.tile([P, P], BF16, tag="m")
                    nc.vector.tensor_tensor(
                        out=mt[:], in0=sp[:],
                        in1=invdeg_bc[:, b * P:(b + 1) * P],
                        op=AluOp.mult)
                    hp = ps_dense.tile([P, P], F32, tag="d")
                    nc.tensor.matmul(out=hp[:],
                                     lhsT=wl_t[:, li * H:(li + 1) * H],
                                     rhs=mt[:], start=True, stop=False)
                    nc.tensor.matmul(out=hp[:],
                                     lhsT=wr_t[:, li * H:(li + 1) * H],
                                     rhs=hT_prev[:, b * P:(b + 1) * P],
                                     start=False, stop=True)
                    pre = prep_.tile([P, P], F32, tag="p")
                    nc.scalar.activation(pre[:], hp[:], Act.Lrelu,
                                         bias=bl_t[:, li:li + 1], scale=1.0,
                                         alpha=NEG_SLOPE)
                    nc.vector.tensor_scalar(
                        out=hT_new[:, b * P:(b + 1) * P], in0=pre[:],
                        scalar1=gs_t[:, li:li + 1], scalar2=bt_t[:, li:li + 1],
                        op0=AluOp.mult, op1=AluOp.add)
                    tp = ps_tr.tile([P, P], BF16, tag="t")
                    nc.tensor.transpose(out=tp[:],
                                        in_=hT_new[:, b * P:(b + 1) * P],
                                        identity=ident_t[:])
                    stg = stgp.tile([P, P], BF16, tag="st")
                    nc.vector.tensor_copy(stg[:], tp[:])
                    if li < NL - 1:
                        nc.sync.dma_start(slices[li][b * P:(b + 1) * P, :],
                                          stg[:])
                    else:
                        gb = gbp.tile([P, NG], BF16, tag="gb")
                        nc.vector.tensor_scalar(
                            out=gb[:], in0=giota_t[:],
                            scalar1=gid_t[:, b:b + 1], scalar2=None,
                            op0=AluOp.is_equal)
                        half = pool_ps[:, :NG] if b < POOL_SPLIT \
                            else pool_ps[:, NG:]
                        nc.tensor.matmul(
                            out=half, lhsT=stg[:], rhs=gb[:],
                            start=(b in (0, POOL_SPLIT)),
                            stop=(b in (POOL_SPLIT - 1, NB - 1)))
                        if b == POOL_SPLIT - 1:
                            poolTa = small.tile([P, NG], F32, tag="poolTa")
                            nc.vector.tensor_copy(poolTa[:], pool_ps[:, :NG])
                            nc.sync.dma_start(payload_a[:], poolTa[:])
                            nc.gpsimd.collective_compute(
                                "AllReduce", AluOp.add, replica_groups=RG,
                                ins=[payload_a[:]], outs=[payload_all_a[:]],
                            )
                if li < NL - 1:
                    nc.gpsimd.collective_compute(
                        "AllGather", AluOp.bypass, replica_groups=RG,
                        ins=[slices[li][:]], outs=[tables[li][:]],
                    )
                hT_prev, hT_new = hT_new, hT_prev
                nc.leave_named_scope(f"layer{li}", _sc[0], False)

            # ---- epilogue ------------------------------------------------
            _sc = nc.enter_named_scope("epilogue", False)
            poolT = small.tile([P, NG], F32, tag="poolT")
            nc.vector.tensor_copy(poolT[:], pool_ps[:, NG:])
            nc.sync.dma_start(payload_b[:], poolT[:])
            nc.gpsimd.collective_compute(
                "AllReduce", AluOp.add, replica_groups=RG,
                ins=[payload_b[:]], outs=[payload_all_b[:]],
            )
            pool_acc = small.tile([P, NG], F32, tag="pacc")
            nc.sync.dma_start(pool_acc[:], payload_all_a[:])
            pool_accb = small.tile([P, NG], F32, tag="paccb")
            nc.sync.dma_start(pool_accb[:], payload_all_b[:])
            nc.vector.tensor_tensor(out=pool_acc[:], in0=pool_acc[:],
                                    in1=pool_accb[:], op=AluOp.add)
            nc.vector.tensor_tensor(out=pool_acc[:], in0=pool_acc[:],
                                    in1=ginv_bc[:], op=AluOp.mult)

            ug_ps = ps_dense.tile([P, NG], F32, tag="d")
            nc.tensor.matmul(out=ug_ps[:], lhsT=wg_t[:], rhs=uT_t[:],
                             start=True, stop=True)
            ugT = small.tile([P, NG], F32, tag="ugT")
            nc.scalar.activation(ugT[:], ug_ps[:], Act.Lrelu, bias=bg_t[:],
                                 scale=1.0, alpha=NEG_SLOPE)

            hid_ps = ps_dense.tile([P, NG], F32, tag="d")
            nc.tensor.matmul(out=hid_ps[:], lhsT=wf1a_t[:],
                             rhs=pool_acc[:], start=True, stop=False)
            nc.tensor.matmul(out=hid_ps[:], lhsT=wf1b_t[:], rhs=ugT[:],
                             start=False, stop=True)
            hidT = small.tile([P, NG], F32, tag="hidT")
            nc.scalar.activation(hidT[:], hid_ps[:], Act.Lrelu, bias=bf1_t[:],
                                 scale=1.0, alpha=NEG_SLOPE)

            y_ps = ps_dense.tile([2, NG], F32, tag="d")
            nc.tensor.matmul(out=y_ps[:], lhsT=wf2_t[:], rhs=hidT[:],
                             start=True, stop=True)
            yT = small.tile([2, NG], F32, tag="yT")
            nc.vector.tensor_scalar(out=yT[:], in0=y_ps[:], scalar1=bf2_t[:],
                                    scalar2=None, op0=AluOp.add)
            nc.sync.dma_start(y_out[:], yT[:])
            nc.leave_named_scope("epilogue", _sc[0], False)

    nc.finalize()
    _legalize_sync_waits(nc)

    common = dict(
        giota=giota_np, ident=ident_np,
        wl=np.asarray(W_l, np.float32).transpose(1, 0, 2)
          .reshape(H, NL * H).astype(BF),
        wr=np.asarray(W_r, np.float32).transpose(1, 0, 2)
          .reshape(H, NL * H).astype(BF),
        bl=np.asarray(b_l, np.float32).T.copy(),
        gs=gscale.T.copy(), bt=beta_np.T.copy(),
        ginv=ginv.reshape(1, NG),
        uT=u.T.copy(),
        wg=np.asarray(W_g, np.float32),
        bg=np.asarray(b_g, np.float32).reshape(P, 1),
        wf1=np.asarray(W_f1, np.float32),
        bf1=np.asarray(b_f1, np.float32).reshape(P, 1),
        wf2=np.asarray(W_f2, np.float32),
        bf2=np.asarray(b_f2, np.float32).reshape(2, 1),
    )
    in_maps = []
    for c in range(NCORES):
        cd = core_data[c]
        in_maps.append(dict(
            common,
            idx_lo=cd["idx_lo"], idx_hi=cd["idx_hi"], sp=cd["sp"],
            pg_lo=cd["pg_lo"], pg_hi=cd["pg_hi"],
            hT0=h0_pad[c * SLICE:(c + 1) * SLICE].T.astype(BF).copy(),
            gid=gids[c],
            invdeg=inv_deg[c * SLICE:(c + 1) * SLICE]
                .reshape(1, SLICE).astype(BF),
        ))

    res = run_bass_kernel_spmd(nc, in_maps, core_ids=list(range(NCORES)),
                               trace=TRACE)
    global LAST_RESULT
    LAST_RESULT = res
    return np.asarray(res.results[0]["y"]).T.astype(np.float32).copy()


TRACE = False
LAST_RESULT = None


# revision 14
# speedup vs baseline: 1.1659x; 1.1659x over previous
"""Trainium2 Bass kernel v3 for MaterialsGraphSAGE.

Changes vs v2 (1713us):
- Host-side node permutation: nodes assigned to (core, block) bins with a
  2D-balanced greedy (lo/hi in-edge counts vs 768 caps) + rank alignment of
  bins across cores -> fewer padded gather tiles (Q7 desc-gen is the
  bottleneck at ~3.4ns/idx, so padded idx cost real time).
- One-hot scatter matrices are pure 0/1 in FP8 (exact) -> half the DMA bytes;
  1/deg is applied in the PSUM->SBUF copy (tensor_tensor mult with a
  broadcast inv-deg row) instead of being folded into the one-hot.
- AllGather of the layer table is split into two chunks so the first chunk
  overlaps the tail of the layer's compute.
- dynamic_dma_scratch_size=32768 (SWDGE ring carveout 2048 descs/queue) to
  let the Q7 run further ahead of the gather DMAs.
"""

import sys

for _p in ("/opt/trn_rl_repo",):
    if _p not in sys.path:
        sys.path.insert(0, _p)

import numpy as np
import ml_dtypes

import concourse.bacc as bacc
import concourse.mybir as mybir
import concourse.tile as tile
from concourse import bass
from concourse.bass_utils import run_bass_kernel_spmd
from concourse.vector_clock import ScopedClock

F32 = mybir.dt.float32
BF16 = mybir.dt.bfloat16
F8E4 = mybir.dt.float8e4
I32 = mybir.dt.int32
I16 = mybir.dt.int16
BF = ml_dtypes.bfloat16
F8 = ml_dtypes.float8_e4m3

P = 128
NCORES = 8
NN = 50000
NG = 256
SLICE = 6272
PADN = SLICE * NCORES      # 50176
NB = SLICE // P            # 49
NL = 4
H = 128
NODE_F = 64
CH = 16                    # gather tiles per call
NEG_SLOPE = 0.01
BN_EPS = 1e-5
CC_SPLIT = 25              # blocks in the first AllGather chunk


# ---------------------------------------------------------------------------
def _patch_tile_drain():
    def _drain_and_barrier(self, tick_clock, wait_clock):
        drain_inst = self.nc.sync.drain()
        wait_clock.add_sem_waits(
            drain_inst.ins, ScopedClock({None: tick_clock.global_clock})
        )
        si = drain_inst.ins.sync_info
        waits = list(si.on_wait) if si is not None else []
        if len(waits) > 1:
            drain_inst.ins.sync_info = mybir.SyncInfo(
                on_wait=[waits[0]], on_update=list(si.on_update)
            )
            for w in waits[1:]:
                extra = self.nc.sync.drain()
                extra.ins.sync_info = mybir.SyncInfo(on_wait=[w], on_update=[])
        self.nc.all_engine_barrier()
        assert self.sems is not None
        popped = self.nc._tile_sem_poison_stack.pop()
        assert popped is self._sem_poison
        self.nc.clear_and_free_semaphores(list(self.sems.allocated().values()))
        self.nc.all_engine_barrier()

    tile.TileContext._drain_and_barrier = _drain_and_barrier


_patch_tile_drain()


def _legalize_sync_waits(nc, max_waits=1):
    for fn in nc.m.functions:
        for bb in fn.blocks:
            out = []
            changed = False
            for ins in bb.instructions:
                si = ins.sync_info
                if si is not None and len(si.on_wait) > max_waits:
                    waits = list(si.on_wait)
                    for w in waits[:-max_waits]:
                        nop = mybir.InstNoOp(
                            name=f"WSPLIT-{nc.next_id()}", ins=[], outs=[]
                        )
                        nop.engine = ins.engine
                        nop.sync_info = mybir.SyncInfo(on_wait=[w], on_update=[])
                        out.append(nop)
                    ins.sync_info = mybir.SyncInfo(
                        on_wait=waits[-max_waits:], on_update=list(si.on_update)
                    )
                    changed = True
                out.append(ins)
            if changed:
                bb.instructions = out


def _lrelu_np(x):
    return np.where(x >= 0, x, np.float32(NEG_SLOPE) * x).astype(np.float32)


def _build_perm(src, dst):
    """Assign each node a position (core, block, slot) so that per-core totals
    and per-(core, block) lo/hi in-edge counts are balanced.  Returns pos[n]."""
    deg = np.bincount(dst, minlength=NN)
    # stage 1: nodes -> cores, LPT on in-degree, node-count cap per core
    order = np.argsort(-deg, kind="stable")
    core_of = np.full(NN, -1, np.int64)
    load = np.zeros(NCORES, np.int64)
    cnt = np.zeros(NCORES, np.int64)
    big = np.int64(1) << 60
    for n in order:
        c = int(np.argmin(np.where(cnt < SLICE, load, big)))
        core_of[n] = c
        load[c] += deg[n]
        cnt[c] += 1
    # halves follow from core assignment (cores 0-3 are the lo half)
    is_hi = core_of[src] >= NCORES // 2
    lo_cnt = np.bincount(dst[~is_hi], minlength=NN).astype(np.int64)
    hi_cnt = np.bincount(dst[is_hi], minlength=NN).astype(np.int64)

    N_FAT = 2
    caps = np.full(NB, 6 * P, np.int64)
    caps[NB - N_FAT:] = 7 * P
    pos = np.full(NN, -1, np.int64)
    for c in range(NCORES):
        nodes = np.where(core_of == c)[0]
        w = lo_cnt[nodes] + hi_cnt[nodes]
        idx = np.argsort(-w, kind="stable")
        lo_f = np.zeros(NB, np.int64)
        hi_f = np.zeros(NB, np.int64)
        n_f = np.zeros(NB, np.int64)
        bin_nodes = [[] for _ in range(NB)]
        for ni in idx:
            n = nodes[ni]
            l, h = lo_cnt[n], hi_cnt[n]
            open_ = n_f < P
            ok = open_ & (lo_f + l <= caps) & (hi_f + h <= caps)
            if ok.any():
                # LPT: feasible bin with smallest max cap-normalized fill
                score = np.where(
                    ok,
                    np.maximum((lo_f + l) * 896 // caps,
                               (hi_f + h) * 896 // caps), big)
                b = int(np.argmin(score))
            else:
                score = np.where(open_, np.maximum(lo_f + l, hi_f + h), big)
                b = int(np.argmin(score))
            bin_nodes[b].append(n)
            lo_f[b] += l
            hi_f[b] += h
            n_f[b] += 1
        # refinement: swap nodes between bins to push every bin under the
        # (CAP, CAP) fill caps where total capacity permits.
        bin_of = np.empty(len(nodes), np.int64)
        node_ix = {int(n): k for k, n in enumerate(nodes)}
        for b in range(NB):
            for n in bin_nodes[b]:
                bin_of[node_ix[int(n)]] = b
        l_arr = lo_cnt[nodes]
        h_arr = hi_cnt[nodes]
        for _sweep in range(12):
            over = [b for b in range(NB)
                    if lo_f[b] > caps[b] or hi_f[b] > caps[b]]
            if not over:
                break
            fixed_any = False
            for bo in over:
                members = np.where(bin_of == bo)[0]
                members = members[np.argsort(-(l_arr[members]
                                               + h_arr[members]))]
                for a in members:
                    cur = (max(lo_f[bo] - caps[bo], 0)
                           + max(hi_f[bo] - caps[bo], 0))
                    if cur == 0:
                        break
                    la, ha = l_arr[a], h_arr[a]
                    new_lo = lo_f[bo] - la + l_arr
                    new_hi = hi_f[bo] - ha + h_arr
                    new_ovf = (np.maximum(new_lo - caps[bo], 0)
                               + np.maximum(new_hi - caps[bo], 0))
                    cap_p = caps[bin_of]
                    ok = (
                        (bin_of != bo)
                        & (lo_f[bin_of] + la - l_arr <= cap_p)
                        & (hi_f[bin_of] + ha - h_arr <= cap_p)
                        & (new_ovf < cur)
                    )
                    cand = np.where(ok)[0]
                    if len(cand) == 0:
                        continue
                    p_ = cand[np.argmin(new_ovf[cand])]
                    b2 = bin_of[p_]
                    dlo, dhi = la - l_arr[p_], ha - h_arr[p_]
                    lo_f[bo] -= dlo; hi_f[bo] -= dhi
                    lo_f[b2] += dlo; hi_f[b2] += dhi
                    bin_of[a], bin_of[p_] = b2, bo
                    fixed_any = True
            if not fixed_any:
                break
        bin_nodes = [[] for _ in range(NB)]
        for k, n in enumerate(nodes):
            bin_nodes[bin_of[k]].append(n)
        # rank-align bins across cores: thin bins sorted by (lo, hi), fat
        # bins pinned at the last ranks on every core.
        order_b = sorted(range(NB), key=lambda b: (caps[b], lo_f[b], hi_f[b]))
        for newb, b in enumerate(order_b):
            for slot, n in enumerate(bin_nodes[b]):
                pos[n] = c * SLICE + newb * P + slot
    assert (pos >= 0).all()
    return pos


def _prepare(src, dst, h0_pad, inv_deg):
    """Per-core edges grouped per (block, src-half) cell (int16 gather limit),
    tile counts normalized across cores. sp/oh columns are block-major; gather
    idx streams and pg are half-major (lo tiles then hi tiles)."""
    HALF = PADN // 2
    per_core = []
    cnts = np.zeros((NCORES, NB, 2), np.int64)
    for c in range(NCORES):
        base = c * SLICE
        m = (dst >= base) & (dst < base + SLICE)
        s_, d_ = src[m], dst[m]
        blk = (d_ - base) >> 7
        half = (s_ >= HALF).astype(np.int64)
        order = np.argsort(blk * 2 + half, kind="stable")
        s_, d_, blk, half = s_[order], d_[order], blk[order], half[order]
        np.add.at(cnts[c], (blk, half), 1)
        per_core.append((s_, d_ - base, blk, half))
    NT2 = np.maximum(0, -(-cnts.max(0) // P))          # [NB, 2]
    for b in range(NB):
        if NT2[b].sum() == 0:
            NT2[b, 0] = 1
    NT_lo, NT_hi = int(NT2[:, 0].sum()), int(NT2[:, 1].sum())
    NTT = NT_lo + NT_hi
    # stream positions (half-major) and block-major columns
    p_of = {}
    pos = [0, 0]
    colbase = np.zeros(NB + 1, np.int64)
    for b in range(NB):
        for h in (0, 1):
            for t in range(int(NT2[b, h])):
                p_of[(b, h, t)] = pos[h]
                pos[h] += 1
        colbase[b + 1] = colbase[b] + NT2[b].sum()

    core_data = []
    for c in range(NCORES):
        s_, doff, blk, half = per_core[c]
        key = blk * 2 + half
        bounds = np.searchsorted(key, np.arange(2 * NB + 1))
        flat = [np.zeros(NT_lo * P, np.int16), np.zeros(NT_hi * P, np.int16)]
        sp = np.zeros((P, NTT, P), F8)
        pg = [np.zeros((P, NT_lo, H), BF), np.zeros((P, NT_hi, H), BF)]
        for b in range(NB):
            col = colbase[b]
            for h in (0, 1):
                lo, hi = bounds[2 * b + h], bounds[2 * b + h + 1]
                sb = s_[lo:hi] - (HALF if h else 0)
                db = doff[lo:hi] - b * P
                n = hi - lo
                for t in range(int(NT2[b, h])):
                    seg = slice(t * P, min((t + 1) * P, n))
                    k = seg.stop - seg.start
                    ph = p_of[(b, h, t)]
                    if k > 0:
                        flat[h][ph * P:ph * P + k] = sb[seg]
                        sp[np.arange(k), col, db[seg]] = F8(1.0)
                        pg[h][:k, ph, :] = h0_pad[sb[seg]
                                                  + (HALF if h else 0)].astype(BF)
                    col += 1
        core_data.append(dict(
            idx_lo=_wrap_idx(flat[0]) if NT_lo else np.zeros((P, 8), np.int16),
            idx_hi=_wrap_idx(flat[1]) if NT_hi else np.zeros((P, 8), np.int16),
            sp=sp.reshape(P, NTT * P),
            pg_lo=pg[0].reshape(P, NT_lo * H),
            pg_hi=pg[1].reshape(P, NT_hi * H)))
    return NT2, colbase, p_of, NT_lo, NT_hi, core_data


def _wrap_idx(flat):
    n = flat.shape[0]
    assert n % 16 == 0
    buf = np.zeros((P, n // 16), np.int16)
    j = np.arange(n)
    for k in range(8):
        buf[16 * k + (j % 16), j // 16] = flat
    return buf


def kernel(x, edge_index, u, batch, W_emb, b_emb, W_l, b_l, W_r, gamma, beta,
           W_g, b_g, W_f1, b_f1, W_f2, b_f2):
    x = np.asarray(x, np.float32)
    u = np.asarray(u, np.float32)
    src0 = np.asarray(edge_index[0], np.int64)
    dst0 = np.asarray(edge_index[1], np.int64)
    batch = np.asarray(batch, np.int64)

    # ---- host precompute ------------------------------------------------
    pos = _build_perm(src0, dst0)
    src = pos[src0]
    dst = pos[dst0]

    h0 = _lrelu_np(x @ np.asarray(W_emb, np.float32)
                   + np.asarray(b_emb, np.float32))        # [NN, H]
    h0_pad = np.zeros((PADN, H), np.float32)
    h0_pad[pos] = h0
    deg = np.bincount(dst, minlength=PADN).astype(np.float32)
    inv_deg = (1.0 / np.maximum(deg, 1.0)).astype(np.float32)
    gcnt = np.bincount(batch, minlength=NG).astype(np.float32)
    ginv = (1.0 / np.maximum(gcnt, 1.0)).astype(np.float32)

    gid_full = np.full(PADN, -1.0, np.float32)
    gid_full[pos] = batch.astype(np.float32)

    NT2, colbase, p_of, NT_lo, NT_hi, core_data = _prepare(src, dst, h0_pad,
                                                           inv_deg)
    NTT = NT_lo + NT_hi
    NTB_MAX = int(NT2.sum(1).max())

    inv_std = np.float32(1.0 / np.sqrt(1.0 + BN_EPS))
    gscale = np.asarray(gamma, np.float32) * inv_std
    beta_np = np.asarray(beta, np.float32)
    BN_FOLDED = bool((gscale > 0).all() and (beta_np == 0).all())
    W_l_eff = np.asarray(W_l, np.float32).copy()
    W_r_eff = np.asarray(W_r, np.float32).copy()
    b_l_eff = np.asarray(b_l, np.float32).copy()
    if BN_FOLDED:
        # lrelu is positively homogeneous: gs*lrelu(x+b) = lrelu(gs*x+gs*b)
        W_l_eff *= gscale[:, None, :]
        W_r_eff *= gscale[:, None, :]
        b_l_eff *= gscale

    giota_np = np.broadcast_to(np.arange(NG, dtype=np.float32), (P, NG)).copy()
    ident_np = np.eye(P, dtype=BF)

    gids = [gid_full[c * SLICE:(c + 1) * SLICE].reshape(NB, P).T.copy()
            for c in range(NCORES)]

    # ------------------------------------------------------------------
    nc = bacc.Bacc(None, num_swdge_queues=4, dynamic_dma_scratch_size=32768)

    def din(name, shape, dtype=F32):
        return nc.dram_tensor(name, shape, dtype, kind="ExternalInput")

    idx_lo_in = din("idx_lo", [P, max(NT_lo, 1) * 8], I16)
    idx_hi_in = din("idx_hi", [P, max(NT_hi, 1) * 8], I16)
    sp_in = din("sp", [P, NTT * P], F8E4)
    pg_lo_in = din("pg_lo", [P, max(NT_lo, 1) * H], BF16)
    pg_hi_in = din("pg_hi", [P, max(NT_hi, 1) * H], BF16)
    hT0_in = din("hT0", [P, SLICE], BF16)
    gid_in = din("gid", [P, NB])
    giota_in = din("giota", [P, NG])
    ident_in = din("ident", [P, P], BF16)
    wl_in = din("wl", [H, NL * H], BF16)
    wr_in = din("wr", [H, NL * H], BF16)
    bl_in = din("bl", [P, NL])
    gs_in = din("gs", [P, NL])
    bt_in = din("bt", [P, NL])
    ginv_in = din("ginv", [1, NG])
    invdeg_in = din("invdeg", [1, SLICE], BF16)
    uT_in = din("uT", [16, NG])
    wg_in = din("wg", [16, H])
    bg_in = din("bg", [P, 1])
    wf1_in = din("wf1", [2 * H, H])
    bf1_in = din("bf1", [P, 1])
    wf2_in = din("wf2", [H, 2])
    bf2_in = din("bf2", [2, 1])
    y_out = nc.dram_tensor("y", [2, NG], F32, kind="ExternalOutput")

    RG = [list(range(NCORES))]
    AluOp = mybir.AluOpType
    Act = mybir.ActivationFunctionType

    # interleaved lo/hi chunk schedule (shared across layers)
    HALF = PADN // 2
    chunks = {0: [], 1: []}
    for h, nt in ((0, NT_lo), (1, NT_hi)):
        for c0 in range(0, nt, CH):
            chunks[h].append((c0, min(CH, nt - c0)))
    inter = []
    i = j = 0
    while i < len(chunks[0]) or j < len(chunks[1]):
        if i < len(chunks[0]):
            inter.append((0,) + chunks[0][i]); i += 1
        if j < len(chunks[1]):
            inter.append((1,) + chunks[1][j]); j += 1

    with tile.TileContext(nc) as tc:
        with (
            tc.tile_pool(name="dram", bufs=1, space="DRAM") as dram,
            tc.tile_pool(name="meta", bufs=1) as meta,
            tc.tile_pool(name="hbuf", bufs=1) as hbuf,
            tc.tile_pool(name="glo", bufs=10) as glo,
            tc.tile_pool(name="ghi", bufs=10) as ghi,
            tc.tile_pool(name="ohp", bufs=5) as ohp,
            tc.tile_pool(name="gbp", bufs=2) as gbp,
            tc.tile_pool(name="mean", bufs=4) as meanp,
            tc.tile_pool(name="pre", bufs=4) as prep_,
            tc.tile_pool(name="stg", bufs=4) as stgp,
            tc.tile_pool(name="ps_scat", bufs=3, space="PSUM") as ps_scat,
            tc.tile_pool(name="ps_dense", bufs=2, space="PSUM") as ps_dense,
            tc.tile_pool(name="ps_tr", bufs=2, space="PSUM") as ps_tr,
            tc.tile_pool(name="ps_pool", bufs=1, space="PSUM") as ps_pool,
            tc.tile_pool(name="small", bufs=2) as small,
        ):
            idx_lo = meta.tile([P, max(NT_lo, 1) * 8], I16)
            idx_hi = meta.tile([P, max(NT_hi, 1) * 8], I16)
            gid_t = meta.tile([P, NB], F32)
            giota_t = meta.tile([P, NG], F32)
            ident_t = meta.tile([P, P], BF16)
            wl_t = meta.tile([H, NL * H], BF16)
            wr_t = meta.tile([H, NL * H], BF16)
            bl_t = meta.tile([P, NL], F32)
            gs_t = meta.tile([P, NL], F32)
            bt_t = meta.tile([P, NL], F32)
            uT_t = meta.tile([16, NG], F32)
            wg_t = meta.tile([16, H], F32)
            bg_t = meta.tile([P, 1], F32)
            wf1a_t = meta.tile([H, H], F32)
            wf1b_t = meta.tile([H, H], F32)
            bf1_t = meta.tile([P, 1], F32)
            wf2_t = meta.tile([H, 2], F32)
            bf2_t = meta.tile([2, 1], F32)
            ginv_bc = meta.tile([P, NG], F32)
            invdeg_bc = meta.tile([P, SLICE], BF16)
            for t_, i_ in (
                (idx_lo, idx_lo_in), (idx_hi, idx_hi_in),
                (gid_t, gid_in), (giota_t, giota_in),
                (ident_t, ident_in), (wl_t, wl_in), (wr_t, wr_in),
                (bl_t, bl_in), (gs_t, gs_in), (bt_t, bt_in),
                (uT_t, uT_in), (wg_t, wg_in), (bg_t, bg_in),
                (wf1a_t, wf1_in[:H, :]), (wf1b_t, wf1_in[H:, :]),
                (bf1_t, bf1_in), (wf2_t, wf2_in), (bf2_t, bf2_in),
            ):
                nc.sync.dma_start(t_[:], i_[:])
            nc.sync.dma_start(ginv_bc[:], ginv_in[:1, :].to_broadcast((P, NG)))
            nc.sync.dma_start(invdeg_bc[:],
                              invdeg_in[:1, :].to_broadcast((P, SLICE)))

            hT_a = hbuf.tile([P, SLICE], BF16, name="hT_a")
            hT_b = hbuf.tile([P, SLICE], BF16, name="hT_b")
            nc.sync.dma_start(hT_a[:], hT0_in[:])

            slices = [dram.tile([SLICE, H], BF16, name=f"slice{i}")
                      for i in range(NL - 1)]
            tables = [dram.tile([PADN, H], BF16, addr_space="Shared",
                                name=f"table{i}") for i in range(NL - 1)]
            payload_a = dram.tile([P, NG], F32, name="payload_a")
            payload_b = dram.tile([P, NG], F32, name="payload_b")
            payload_all_a = dram.tile([P, NG], F32, addr_space="Shared",
                                      name="payload_all_a")
            payload_all_b = dram.tile([P, NG], F32, addr_space="Shared",
                                      name="payload_all_b")

            hT_prev, hT_new = hT_a, hT_b
            pool_ps = ps_pool.tile([P, 2 * NG], F32, tag="pp")
            POOL_SPLIT = 25

            for li in range(NL):
                _sc = nc.enter_named_scope(f"layer{li}", False)
                chunk_tiles = {0: [], 1: []}
                qrr = 0
                for h, c0, ntc in inter:
                    g = (glo if h == 0 else ghi).tile([P, CH, H], BF16, tag="g")
                    if li == 0:
                        pgsrc = pg_lo_in if h == 0 else pg_hi_in
                        nc.sync.dma_start(
                            g[:, :ntc, :].rearrange("p a b -> p (a b)"),
                            pgsrc[:, c0 * H:(c0 + ntc) * H])
                    else:
                        idx_t_ = idx_lo if h == 0 else idx_hi
                        nidx = ntc * P
                        f0 = c0 * P // 16
                        nc.gpsimd.dma_gather(
                            out_ap=g[:, :ntc, :],
                            in_ap=tables[li - 1][h * HALF:(h + 1) * HALF, :],
                            idxs_ap=idx_t_[:, f0:f0 + nidx // 16],
                            num_idxs=nidx, num_idxs_reg=nidx, elem_size=H,
                            queue_num=qrr % 4, single_packet=False)
                        qrr += 1
                    chunk_tiles[h].append(g)

                for b in range(NB):
                    refs = ([(0, t) for t in range(int(NT2[b, 0]))]
                            + [(1, t) for t in range(int(NT2[b, 1]))])
                    ntb = len(refs)
                    col0 = int(colbase[b])
                    oh = ohp.tile([P, NTB_MAX * P], F8E4, tag="oh")
                    nc.sync.dma_start(oh[:, :ntb * P],
                                      sp_in[:, col0 * P:(col0 + ntb) * P])
                    sp = ps_scat.tile([P, P], F32, tag="sc")
                    for i_r, (h, t) in enumerate(refs):
                        ph = p_of[(b, h, t)]
                        g = chunk_tiles[h][ph // CH]
                        nc.tensor.matmul(out=sp[:],
                                         lhsT=g[:, ph % CH, :],
                                         rhs=oh[:, i_r * P:(i_r + 1) * P],
                                         start=(i_r == 0),
                                         stop=(i_r == ntb - 1))
                    mt = meanp.tile([P, P], BF16, tag="m")
                    nc.vector.tensor_tensor(
                        out=mt[:], in0=sp[:],
                        in1=invdeg_bc[:, b * P:(b + 1) * P],
                        op=AluOp.mult)
                    hp = ps_dense.tile([P, P], F32, tag="d")
                    nc.tensor.matmul(out=hp[:],
                                     lhsT=wl_t[:, li * H:(li + 1) * H],
                                     rhs=mt[:], start=True, stop=False)
                    nc.tensor.matmul(out=hp[:],
                                     lhsT=wr_t[:, li * H:(li + 1) * H],
                                     rhs=hT_prev[:, b * P:(b + 1) * P],
                                     start=False, stop=True)
                    if BN_FOLDED:
                        nc.scalar.activation(
                            hT_new[:, b * P:(b + 1) * P], hp[:], Act.Lrelu,
                            bias=bl_t[:, li:li + 1], scale=1.0,
                            alpha=NEG_SLOPE)
                    else:
                        pre = prep_.tile([P, P], F32, tag="p")
                        nc.scalar.activation(pre[:], hp[:], Act.Lrelu,
                                             bias=bl_t[:, li:li + 1],
                                             scale=1.0, alpha=NEG_SLOPE)
                        nc.vector.tensor_scalar(
                            out=hT_new[:, b * P:(b + 1) * P], in0=pre[:],
                            scalar1=gs_t[:, li:li + 1],
                            scalar2=bt_t[:, li:li + 1],
                            op0=AluOp.mult, op1=AluOp.add)
                    tp = ps_tr.tile([P, P], BF16, tag="t")
                    nc.tensor.transpose(out=tp[:],
                                        in_=hT_new[:, b * P:(b + 1) * P],
                                        identity=ident_t[:])
                    stg = stgp.tile([P, P], BF16, tag="st")
                    nc.vector.tensor_copy(stg[:], tp[:])
                    if li < NL - 1:
                        nc.sync.dma_start(slices[li][b * P:(b + 1) * P, :],
                                          stg[:])
                    else:
                        gb = gbp.tile([P, NG], BF16, tag="gb")
                        nc.vector.tensor_scalar(
                            out=gb[:], in0=giota_t[:],
                            scalar1=gid_t[:, b:b + 1], scalar2=None,
                            op0=AluOp.is_equal)
                        half = pool_ps[:, :NG] if b < POOL_SPLIT \
                            else pool_ps[:, NG:]
                        nc.tensor.matmul(
                            out=half, lhsT=stg[:], rhs=gb[:],
                            start=(b in (0, POOL_SPLIT)),
                            stop=(b in (POOL_SPLIT - 1, NB - 1)))
                        if b == POOL_SPLIT - 1:
                            poolTa = small.tile([P, NG], F32, tag="poolTa")
                            nc.vector.tensor_copy(poolTa[:], pool_ps[:, :NG])
                            nc.sync.dma_start(payload_a[:], poolTa[:])
                            nc.gpsimd.collective_compute(
                                "AllReduce", AluOp.add, replica_groups=RG,
                                ins=[payload_a[:]], outs=[payload_all_a[:]],
                            )
                if li < NL - 1:
                    nc.gpsimd.collective_compute(
                        "AllGather", AluOp.bypass, replica_groups=RG,
                        ins=[slices[li][:]], outs=[tables[li][:]],
                    )
                hT_prev, hT_new = hT_new, hT_prev
                nc.leave_named_scope(f"layer{li}", _sc[0], False)

            # ---- epilogue ------------------------------------------------
            _sc = nc.enter_named_scope("epilogue", False)
            poolT = small.tile([P, NG], F32, tag="poolT")
            nc.vector.tensor_copy(poolT[:], pool_ps[:, NG:])
            nc.sync.dma_start(payload_b[:], poolT[:])
            nc.gpsimd.collective_compute(
                "AllReduce", AluOp.add, replica_groups=RG,
                ins=[payload_b[:]], outs=[payload_all_b[:]],
            )
            pool_acc = small.tile([P, NG], F32, tag="pacc")
            nc.sync.dma_start(pool_acc[:], payload_all_a[:])
            pool_accb = small.tile([P, NG], F32, tag="paccb")
            nc.sync.dma_start(pool_accb[:], payload_all_b[:])
            nc.vector.tensor_tensor(out=pool_acc[:], in0=pool_acc[:],
                                    in1=pool_accb[:], op=AluOp.add)
            nc.vector.tensor_tensor(out=pool_acc[:], in0=pool_acc[:],
                                    in1=ginv_bc[:], op=AluOp.mult)

            ug_ps = ps_dense.tile([P, NG], F32, tag="d")
            nc.tensor.matmul(out=ug_ps[:], lhsT=wg_t[:], rhs=uT_t[:],
                             start=True, stop=True)
            ugT = small.tile([P, NG], F32, tag="ugT")
            nc.scalar.activation(ugT[:], ug_ps[:], Act.Lrelu, bias=bg_t[:],
                                 scale=1.0, alpha=NEG_SLOPE)

            hid_ps = ps_dense.tile([P, NG], F32, tag="d")
            nc.tensor.matmul(out=hid_ps[:], lhsT=wf1a_t[:],
                             rhs=pool_acc[:], start=True, stop=False)
            nc.tensor.matmul(out=hid_ps[:], lhsT=wf1b_t[:], rhs=ugT[:],
                             start=False, stop=True)
            hidT = small.tile([P, NG], F32, tag="hidT")
            nc.scalar.activation(hidT[:], hid_ps[:], Act.Lrelu, bias=bf1_t[:],
                                 scale=1.0, alpha=NEG_SLOPE)

            y_ps = ps_dense.tile([2, NG], F32, tag="d")
            nc.tensor.matmul(out=y_ps[:], lhsT=wf2_t[:], rhs=hidT[:],
                             start=True, stop=True)
            yT = small.tile([2, NG], F32, tag="yT")
            nc.vector.tensor_scalar(out=yT[:], in0=y_ps[:], scalar1=bf2_t[:],
                                    scalar2=None, op0=AluOp.add)
            nc.sync.dma_start(y_out[:], yT[:])
            nc.leave_named_scope("epilogue", _sc[0], False)

    nc.finalize()
    _legalize_sync_waits(nc)

    common = dict(
        giota=giota_np, ident=ident_np,
        wl=W_l_eff.transpose(1, 0, 2).reshape(H, NL * H).astype(BF),
        wr=W_r_eff.transpose(1, 0, 2).reshape(H, NL * H).astype(BF),
        bl=b_l_eff.T.copy(),
        gs=gscale.T.copy(), bt=beta_np.T.copy(),
        ginv=ginv.reshape(1, NG),
        uT=u.T.copy(),
        wg=np.asarray(W_g, np.float32),
        bg=np.asarray(b_g, np.float32).reshape(P, 1),
        wf1=np.asarray(W_f1, np.float32),
        bf1=np.asarray(b_f1, np.float32).reshape(P, 1),
        wf2=np.asarray(W_f2, np.float32),
        bf2=np.asarray(b_f2, np.float32).reshape(2, 1),
    )
    in_maps = []
    for c in range(NCORES):
        cd = core_data[c]
        in_maps.append(dict(
            common,
            idx_lo=cd["idx_lo"], idx_hi=cd["idx_hi"], sp=cd["sp"],
            pg_lo=cd["pg_lo"], pg_hi=cd["pg_hi"],
            hT0=h0_pad[c * SLICE:(c + 1) * SLICE].T.astype(BF).copy(),
            gid=gids[c],
            invdeg=inv_deg[c * SLICE:(c + 1) * SLICE]
                .reshape(1, SLICE).astype(BF),
        ))

    res = run_bass_kernel_spmd(nc, in_maps, core_ids=list(range(NCORES)),
                               trace=TRACE)
    global LAST_RESULT
    LAST_RESULT = res
    return np.asarray(res.results[0]["y"]).T.astype(np.float32).copy()


TRACE = False
LAST_RESULT = None


# revision 15
# speedup vs baseline: 1.1699x; 1.0035x over previous
"""Trainium2 Bass kernel v3 for MaterialsGraphSAGE.

Changes vs v2 (1713us):
- Host-side node permutation: nodes assigned to (core, block) bins with a
  2D-balanced greedy (lo/hi in-edge counts vs 768 caps) + rank alignment of
  bins across cores -> fewer padded gather tiles (Q7 desc-gen is the
  bottleneck at ~3.4ns/idx, so padded idx cost real time).
- One-hot scatter matrices are pure 0/1 in FP8 (exact) -> half the DMA bytes;
  1/deg is applied in the PSUM->SBUF copy (tensor_tensor mult with a
  broadcast inv-deg row) instead of being folded into the one-hot.
- AllGather of the layer table is split into two chunks so the first chunk
  overlaps the tail of the layer's compute.
- dynamic_dma_scratch_size=32768 (SWDGE ring carveout 2048 descs/queue) to
  let the Q7 run further ahead of the gather DMAs.
"""

import sys

for _p in ("/opt/trn_rl_repo",):
    if _p not in sys.path:
        sys.path.insert(0, _p)

import numpy as np
import ml_dtypes

import concourse.bacc as bacc
import concourse.mybir as mybir
import concourse.tile as tile
from concourse import bass
from concourse.bass_utils import run_bass_kernel_spmd
from concourse.vector_clock import ScopedClock

F32 = mybir.dt.float32
BF16 = mybir.dt.bfloat16
F8E4 = mybir.dt.float8e4
I32 = mybir.dt.int32
I16 = mybir.dt.int16
BF = ml_dtypes.bfloat16
F8 = ml_dtypes.float8_e4m3

P = 128
NCORES = 8
NN = 50000
NG = 256
SLICE = 6272
PADN = SLICE * NCORES      # 50176
NB = SLICE // P            # 49
NL = 4
H = 128
NODE_F = 64
CH = 16                    # gather tiles per call
NEG_SLOPE = 0.01
BN_EPS = 1e-5
CC_SPLIT = 25              # blocks in the first AllGather chunk


# ---------------------------------------------------------------------------
def _patch_tile_drain():
    def _drain_and_barrier(self, tick_clock, wait_clock):
        drain_inst = self.nc.sync.drain()
        wait_clock.add_sem_waits(
            drain_inst.ins, ScopedClock({None: tick_clock.global_clock})
        )
        si = drain_inst.ins.sync_info
        waits = list(si.on_wait) if si is not None else []
        if len(waits) > 1:
            drain_inst.ins.sync_info = mybir.SyncInfo(
                on_wait=[waits[0]], on_update=list(si.on_update)
            )
            for w in waits[1:]:
                extra = self.nc.sync.drain()
                extra.ins.sync_info = mybir.SyncInfo(on_wait=[w], on_update=[])
        self.nc.all_engine_barrier()
        assert self.sems is not None
        popped = self.nc._tile_sem_poison_stack.pop()
        assert popped is self._sem_poison
        self.nc.clear_and_free_semaphores(list(self.sems.allocated().values()))
        self.nc.all_engine_barrier()

    tile.TileContext._drain_and_barrier = _drain_and_barrier


_patch_tile_drain()


def _legalize_sync_waits(nc, max_waits=1):
    for fn in nc.m.functions:
        for bb in fn.blocks:
            out = []
            changed = False
            for ins in bb.instructions:
                si = ins.sync_info
                if si is not None and len(si.on_wait) > max_waits:
                    waits = list(si.on_wait)
                    for w in waits[:-max_waits]:
                        nop = mybir.InstNoOp(
                            name=f"WSPLIT-{nc.next_id()}", ins=[], outs=[]
                        )
                        nop.engine = ins.engine
                        nop.sync_info = mybir.SyncInfo(on_wait=[w], on_update=[])
                        out.append(nop)
                    ins.sync_info = mybir.SyncInfo(
                        on_wait=waits[-max_waits:], on_update=list(si.on_update)
                    )
                    changed = True
                out.append(ins)
            if changed:
                bb.instructions = out


def _lrelu_np(x):
    return np.where(x >= 0, x, np.float32(NEG_SLOPE) * x).astype(np.float32)


def _build_perm(src, dst):
    """Assign each node a position (core, block, slot) so that per-core totals
    and per-(core, block) lo/hi in-edge counts are balanced.  Returns pos[n]."""
    deg = np.bincount(dst, minlength=NN)
    # stage 1: nodes -> cores, LPT on in-degree, node-count cap per core
    order = np.argsort(-deg, kind="stable")
    core_of = np.full(NN, -1, np.int64)
    load = np.zeros(NCORES, np.int64)
    cnt = np.zeros(NCORES, np.int64)
    big = np.int64(1) << 60
    for n in order:
        c = int(np.argmin(np.where(cnt < SLICE, load, big)))
        core_of[n] = c
        load[c] += deg[n]
        cnt[c] += 1
    # halves follow from core assignment (cores 0-3 are the lo half)
    is_hi = core_of[src] >= NCORES // 2
    lo_cnt = np.bincount(dst[~is_hi], minlength=NN).astype(np.int64)
    hi_cnt = np.bincount(dst[is_hi], minlength=NN).astype(np.int64)

    N_FAT = 2
    caps = np.full(NB, 6 * P, np.int64)
    caps[NB - N_FAT:] = 7 * P
    pos = np.full(NN, -1, np.int64)
    for c in range(NCORES):
        nodes = np.where(core_of == c)[0]
        w = lo_cnt[nodes] + hi_cnt[nodes]
        idx = np.argsort(-w, kind="stable")
        lo_f = np.zeros(NB, np.int64)
        hi_f = np.zeros(NB, np.int64)
        n_f = np.zeros(NB, np.int64)
        bin_nodes = [[] for _ in range(NB)]
        for ni in idx:
            n = nodes[ni]
            l, h = lo_cnt[n], hi_cnt[n]
            open_ = n_f < P
            ok = open_ & (lo_f + l <= caps) & (hi_f + h <= caps)
            if ok.any():
                # LPT: feasible bin with smallest max cap-normalized fill
                score = np.where(
                    ok,
                    np.maximum((lo_f + l) * 896 // caps,
                               (hi_f + h) * 896 // caps), big)
                b = int(np.argmin(score))
            else:
                score = np.where(open_, np.maximum(lo_f + l, hi_f + h), big)
                b = int(np.argmin(score))
            bin_nodes[b].append(n)
            lo_f[b] += l
            hi_f[b] += h
            n_f[b] += 1
        # refinement: swap nodes between bins to push every bin under the
        # (CAP, CAP) fill caps where total capacity permits.
        bin_of = np.empty(len(nodes), np.int64)
        node_ix = {int(n): k for k, n in enumerate(nodes)}
        for b in range(NB):
            for n in bin_nodes[b]:
                bin_of[node_ix[int(n)]] = b
        l_arr = lo_cnt[nodes]
        h_arr = hi_cnt[nodes]
        for _sweep in range(12):
            over = [b for b in range(NB)
                    if lo_f[b] > caps[b] or hi_f[b] > caps[b]]
            if not over:
                break
            fixed_any = False
            for bo in over:
                members = np.where(bin_of == bo)[0]
                members = members[np.argsort(-(l_arr[members]
                                               + h_arr[members]))]
                for a in members:
                    cur = (max(lo_f[bo] - caps[bo], 0)
                           + max(hi_f[bo] - caps[bo], 0))
                    if cur == 0:
                        break
                    la, ha = l_arr[a], h_arr[a]
                    new_lo = lo_f[bo] - la + l_arr
                    new_hi = hi_f[bo] - ha + h_arr
                    new_ovf = (np.maximum(new_lo - caps[bo], 0)
                               + np.maximum(new_hi - caps[bo], 0))
                    cap_p = caps[bin_of]
                    ok = (
                        (bin_of != bo)
                        & (lo_f[bin_of] + la - l_arr <= cap_p)
                        & (hi_f[bin_of] + ha - h_arr <= cap_p)
                        & (new_ovf < cur)
                    )
                    cand = np.where(ok)[0]
                    if len(cand) == 0:
                        continue
                    p_ = cand[np.argmin(new_ovf[cand])]
                    b2 = bin_of[p_]
                    dlo, dhi = la - l_arr[p_], ha - h_arr[p_]
                    lo_f[bo] -= dlo; hi_f[bo] -= dhi
                    lo_f[b2] += dlo; hi_f[b2] += dhi
                    bin_of[a], bin_of[p_] = b2, bo
                    fixed_any = True
            if not fixed_any:
                break
        bin_nodes = [[] for _ in range(NB)]
        for k, n in enumerate(nodes):
            bin_nodes[bin_of[k]].append(n)
        # rank-align bins across cores: thin bins sorted by (lo, hi), fat
        # bins pinned at the last ranks on every core.
        order_b = sorted(range(NB), key=lambda b: (caps[b], lo_f[b], hi_f[b]))
        for newb, b in enumerate(order_b):
            for slot, n in enumerate(bin_nodes[b]):
                pos[n] = c * SLICE + newb * P + slot
    assert (pos >= 0).all()
    return pos


def _prepare(src, dst, h0_pad, inv_deg):
    """Per-core edges grouped per (block, src-half) cell (int16 gather limit),
    tile counts normalized across cores. sp/oh columns are block-major; gather
    idx streams and pg are half-major (lo tiles then hi tiles)."""
    HALF = PADN // 2
    per_core = []
    cnts = np.zeros((NCORES, NB, 2), np.int64)
    for c in range(NCORES):
        base = c * SLICE
        m = (dst >= base) & (dst < base + SLICE)
        s_, d_ = src[m], dst[m]
        blk = (d_ - base) >> 7
        half = (s_ >= HALF).astype(np.int64)
        order = np.argsort(blk * 2 + half, kind="stable")
        s_, d_, blk, half = s_[order], d_[order], blk[order], half[order]
        np.add.at(cnts[c], (blk, half), 1)
        per_core.append((s_, d_ - base, blk, half))
    NT2 = np.maximum(0, -(-cnts.max(0) // P))          # [NB, 2]
    for b in range(NB):
        if NT2[b].sum() == 0:
            NT2[b, 0] = 1
    NT_lo, NT_hi = int(NT2[:, 0].sum()), int(NT2[:, 1].sum())
    NTT = NT_lo + NT_hi
    # stream positions (half-major) and block-major columns
    p_of = {}
    pos = [0, 0]
    colbase = np.zeros(NB + 1, np.int64)
    for b in range(NB):
        for h in (0, 1):
            for t in range(int(NT2[b, h])):
                p_of[(b, h, t)] = pos[h]
                pos[h] += 1
        colbase[b + 1] = colbase[b] + NT2[b].sum()

    core_data = []
    for c in range(NCORES):
        s_, doff, blk, half = per_core[c]
        key = blk * 2 + half
        bounds = np.searchsorted(key, np.arange(2 * NB + 1))
        flat = [np.zeros(NT_lo * P, np.int16), np.zeros(NT_hi * P, np.int16)]
        sp = np.zeros((P, NTT, P), F8)
        pg = [np.zeros((P, NT_lo, H), BF), np.zeros((P, NT_hi, H), BF)]
        for b in range(NB):
            col = colbase[b]
            for h in (0, 1):
                lo, hi = bounds[2 * b + h], bounds[2 * b + h + 1]
                sb = s_[lo:hi] - (HALF if h else 0)
                db = doff[lo:hi] - b * P
                n = hi - lo
                for t in range(int(NT2[b, h])):
                    seg = slice(t * P, min((t + 1) * P, n))
                    k = seg.stop - seg.start
                    ph = p_of[(b, h, t)]
                    if k > 0:
                        flat[h][ph * P:ph * P + k] = sb[seg]
                        sp[np.arange(k), col, db[seg]] = F8(1.0)
                        pg[h][:k, ph, :] = h0_pad[sb[seg]
                                                  + (HALF if h else 0)].astype(BF)
                    col += 1
        core_data.append(dict(
            idx_lo=_wrap_idx(flat[0]) if NT_lo else np.zeros((P, 8), np.int16),
            idx_hi=_wrap_idx(flat[1]) if NT_hi else np.zeros((P, 8), np.int16),
            sp=sp.reshape(P, NTT * P),
            pg_lo=pg[0].reshape(P, NT_lo * H),
            pg_hi=pg[1].reshape(P, NT_hi * H)))
    return NT2, colbase, p_of, NT_lo, NT_hi, core_data


def _wrap_idx(flat):
    n = flat.shape[0]
    assert n % 16 == 0
    buf = np.zeros((P, n // 16), np.int16)
    j = np.arange(n)
    for k in range(8):
        buf[16 * k + (j % 16), j // 16] = flat
    return buf


def kernel(x, edge_index, u, batch, W_emb, b_emb, W_l, b_l, W_r, gamma, beta,
           W_g, b_g, W_f1, b_f1, W_f2, b_f2):
    x = np.asarray(x, np.float32)
    u = np.asarray(u, np.float32)
    src0 = np.asarray(edge_index[0], np.int64)
    dst0 = np.asarray(edge_index[1], np.int64)
    batch = np.asarray(batch, np.int64)

    # ---- host precompute ------------------------------------------------
    pos = _build_perm(src0, dst0)
    src = pos[src0]
    dst = pos[dst0]

    h0 = _lrelu_np(x @ np.asarray(W_emb, np.float32)
                   + np.asarray(b_emb, np.float32))        # [NN, H]
    h0_pad = np.zeros((PADN, H), np.float32)
    h0_pad[pos] = h0
    deg = np.bincount(dst, minlength=PADN).astype(np.float32)
    inv_deg = (1.0 / np.maximum(deg, 1.0)).astype(np.float32)
    gcnt = np.bincount(batch, minlength=NG).astype(np.float32)
    ginv = (1.0 / np.maximum(gcnt, 1.0)).astype(np.float32)

    gid_full = np.full(PADN, -1.0, np.float32)
    gid_full[pos] = batch.astype(np.float32)

    NT2, colbase, p_of, NT_lo, NT_hi, core_data = _prepare(src, dst, h0_pad,
                                                           inv_deg)
    NTT = NT_lo + NT_hi
    NTB_MAX = int(NT2.sum(1).max())

    inv_std = np.float32(1.0 / np.sqrt(1.0 + BN_EPS))
    gscale = np.asarray(gamma, np.float32) * inv_std
    beta_np = np.asarray(beta, np.float32)
    BN_FOLDED = bool((gscale > 0).all() and (beta_np == 0).all())
    W_l_eff = np.asarray(W_l, np.float32).copy()
    W_r_eff = np.asarray(W_r, np.float32).copy()
    b_l_eff = np.asarray(b_l, np.float32).copy()
    if BN_FOLDED:
        # lrelu is positively homogeneous: gs*lrelu(x+b) = lrelu(gs*x+gs*b)
        W_l_eff *= gscale[:, None, :]
        W_r_eff *= gscale[:, None, :]
        b_l_eff *= gscale

    giota_np = np.broadcast_to(np.arange(NG, dtype=np.float32), (P, NG)).copy()
    ident_np = np.eye(P, dtype=BF)

    gids = [gid_full[c * SLICE:(c + 1) * SLICE].reshape(NB, P).T.copy()
            for c in range(NCORES)]

    # ------------------------------------------------------------------
    nc = bacc.Bacc(None, num_swdge_queues=4, dynamic_dma_scratch_size=32768)

    def din(name, shape, dtype=F32):
        return nc.dram_tensor(name, shape, dtype, kind="ExternalInput")

    idx_lo_in = din("idx_lo", [P, max(NT_lo, 1) * 8], I16)
    idx_hi_in = din("idx_hi", [P, max(NT_hi, 1) * 8], I16)
    sp_in = din("sp", [P, NTT * P], F8E4)
    pg_lo_in = din("pg_lo", [P, max(NT_lo, 1) * H], BF16)
    pg_hi_in = din("pg_hi", [P, max(NT_hi, 1) * H], BF16)
    hT0_in = din("hT0", [P, SLICE], BF16)
    gid_in = din("gid", [P, NB])
    giota_in = din("giota", [P, NG])
    ident_in = din("ident", [P, P], BF16)
    wl_in = din("wl", [H, NL * H], BF16)
    wr_in = din("wr", [H, NL * H], BF16)
    bl_in = din("bl", [P, NL])
    gs_in = din("gs", [P, NL])
    bt_in = din("bt", [P, NL])
    ginv_in = din("ginv", [1, NG])
    invdeg_in = din("invdeg", [1, SLICE], BF16)
    uT_in = din("uT", [16, NG])
    wg_in = din("wg", [16, H])
    bg_in = din("bg", [P, 1])
    wf1_in = din("wf1", [2 * H, H])
    bf1_in = din("bf1", [P, 1])
    wf2_in = din("wf2", [H, 2])
    bf2_in = din("bf2", [2, 1])
    y_out = nc.dram_tensor("y", [2, NG], F32, kind="ExternalOutput")

    RG = [list(range(NCORES))]
    AluOp = mybir.AluOpType
    Act = mybir.ActivationFunctionType

    # interleaved lo/hi chunk schedule (shared across layers)
    HALF = PADN // 2
    chunks = {0: [], 1: []}
    for h, nt in ((0, NT_lo), (1, NT_hi)):
        for c0 in range(0, nt, CH):
            chunks[h].append((c0, min(CH, nt - c0)))
    inter = []
    i = j = 0
    while i < len(chunks[0]) or j < len(chunks[1]):
        if i < len(chunks[0]):
            inter.append((0,) + chunks[0][i]); i += 1
        if j < len(chunks[1]):
            inter.append((1,) + chunks[1][j]); j += 1

    with tile.TileContext(nc) as tc:
        with (
            tc.tile_pool(name="dram", bufs=1, space="DRAM") as dram,
            tc.tile_pool(name="meta", bufs=1) as meta,
            tc.tile_pool(name="hbuf", bufs=1) as hbuf,
            tc.tile_pool(name="glo", bufs=10) as glo,
            tc.tile_pool(name="ghi", bufs=10) as ghi,
            tc.tile_pool(name="ohp", bufs=5) as ohp,
            tc.tile_pool(name="gbp", bufs=2) as gbp,
            tc.tile_pool(name="mean", bufs=4) as meanp,
            tc.tile_pool(name="pre", bufs=4) as prep_,
            tc.tile_pool(name="stg", bufs=4) as stgp,
            tc.tile_pool(name="ps_scat", bufs=3, space="PSUM") as ps_scat,
            tc.tile_pool(name="ps_dense", bufs=2, space="PSUM") as ps_dense,
            tc.tile_pool(name="ps_tr", bufs=2, space="PSUM") as ps_tr,
            tc.tile_pool(name="ps_pool", bufs=1, space="PSUM") as ps_pool,
            tc.tile_pool(name="small", bufs=2) as small,
        ):
            idx_lo = meta.tile([P, max(NT_lo, 1) * 8], I16)
            idx_hi = meta.tile([P, max(NT_hi, 1) * 8], I16)
            gid_t = meta.tile([P, NB], F32)
            giota_t = meta.tile([P, NG], F32)
            ident_t = meta.tile([P, P], BF16)
            wl_t = meta.tile([H, NL * H], BF16)
            wr_t = meta.tile([H, NL * H], BF16)
            bl_t = meta.tile([P, NL], F32)
            gs_t = meta.tile([P, NL], F32)
            bt_t = meta.tile([P, NL], F32)
            uT_t = meta.tile([16, NG], F32)
            wg_t = meta.tile([16, H], F32)
            bg_t = meta.tile([P, 1], F32)
            wf1a_t = meta.tile([H, H], F32)
            wf1b_t = meta.tile([H, H], F32)
            bf1_t = meta.tile([P, 1], F32)
            wf2_t = meta.tile([H, 2], F32)
            bf2_t = meta.tile([2, 1], F32)
            ginv_bc = meta.tile([P, NG], F32)
            invdeg_bc = meta.tile([P, SLICE], BF16)
            for t_, i_ in (
                (idx_lo, idx_lo_in), (idx_hi, idx_hi_in),
                (gid_t, gid_in), (giota_t, giota_in),
                (ident_t, ident_in), (wl_t, wl_in), (wr_t, wr_in),
                (bl_t, bl_in), (gs_t, gs_in), (bt_t, bt_in),
                (uT_t, uT_in), (wg_t, wg_in), (bg_t, bg_in),
                (wf1a_t, wf1_in[:H, :]), (wf1b_t, wf1_in[H:, :]),
                (bf1_t, bf1_in), (wf2_t, wf2_in), (bf2_t, bf2_in),
            ):
                nc.sync.dma_start(t_[:], i_[:])
            nc.sync.dma_start(ginv_bc[:], ginv_in[:1, :].to_broadcast((P, NG)))
            nc.sync.dma_start(invdeg_bc[:],
                              invdeg_in[:1, :].to_broadcast((P, SLICE)))

            hT_a = hbuf.tile([P, SLICE], BF16, name="hT_a")
            hT_b = hbuf.tile([P, SLICE], BF16, name="hT_b")
            nc.sync.dma_start(hT_a[:], hT0_in[:])

            slices = [dram.tile([SLICE, H], BF16, name=f"slice{i}")
                      for i in range(NL - 1)]
            tables = [dram.tile([PADN, H], BF16, addr_space="Shared",
                                name=f"table{i}") for i in range(NL - 1)]
            payload_a = dram.tile([P, NG], F32, name="payload_a")
            payload_b = dram.tile([P, NG], F32, name="payload_b")
            payload_all_a = dram.tile([P, NG], F32, addr_space="Shared",
                                      name="payload_all_a")
            payload_all_b = dram.tile([P, NG], F32, addr_space="Shared",
                                      name="payload_all_b")

            hT_prev, hT_new = hT_a, hT_b
            pool_ps = ps_pool.tile([P, 2 * NG], F32, tag="pp")
            POOL_SPLIT = 25

            for li in range(NL):
                _sc = nc.enter_named_scope(f"layer{li}", False)
                chunk_tiles = {0: [], 1: []}
                qrr = 0
                for h, c0, ntc in inter:
                    g = (glo if h == 0 else ghi).tile([P, CH, H], BF16, tag="g")
                    if li == 0:
                        pgsrc = pg_lo_in if h == 0 else pg_hi_in
                        nc.sync.dma_start(
                            g[:, :ntc, :].rearrange("p a b -> p (a b)"),
                            pgsrc[:, c0 * H:(c0 + ntc) * H])
                    else:
                        idx_t_ = idx_lo if h == 0 else idx_hi
                        nidx = ntc * P
                        f0 = c0 * P // 16
                        nc.gpsimd.dma_gather(
                            out_ap=g[:, :ntc, :],
                            in_ap=tables[li - 1][h * HALF:(h + 1) * HALF, :],
                            idxs_ap=idx_t_[:, f0:f0 + nidx // 16],
                            num_idxs=nidx, num_idxs_reg=nidx, elem_size=H,
                            queue_num=qrr % 4, single_packet=False)
                        qrr += 1
                    chunk_tiles[h].append(g)

                for b in range(NB):
                    refs = ([(0, t) for t in range(int(NT2[b, 0]))]
                            + [(1, t) for t in range(int(NT2[b, 1]))])
                    ntb = len(refs)
                    col0 = int(colbase[b])
                    oh = ohp.tile([P, NTB_MAX * P], F8E4, tag="oh")
                    nc.sync.dma_start(oh[:, :ntb * P],
                                      sp_in[:, col0 * P:(col0 + ntb) * P])
                    sp = ps_scat.tile([P, P], F32, tag="sc")
                    for i_r, (h, t) in enumerate(refs):
                        ph = p_of[(b, h, t)]
                        g = chunk_tiles[h][ph // CH]
                        nc.tensor.matmul(out=sp[:],
                                         lhsT=g[:, ph % CH, :],
                                         rhs=oh[:, i_r * P:(i_r + 1) * P],
                                         start=(i_r == 0),
                                         stop=(i_r == ntb - 1))
                    mt = meanp.tile([P, P], BF16, tag="m")
                    nc.vector.tensor_tensor(
                        out=mt[:], in0=sp[:],
                        in1=invdeg_bc[:, b * P:(b + 1) * P],
                        op=AluOp.mult)
                    hp = ps_dense.tile([P, P], F32, tag="d")
                    nc.tensor.matmul(out=hp[:],
                                     lhsT=wl_t[:, li * H:(li + 1) * H],
                                     rhs=mt[:], start=True, stop=False)
                    nc.tensor.matmul(out=hp[:],
                                     lhsT=wr_t[:, li * H:(li + 1) * H],
                                     rhs=hT_prev[:, b * P:(b + 1) * P],
                                     start=False, stop=True)
                    if BN_FOLDED:
                        nc.scalar.activation(
                            hT_new[:, b * P:(b + 1) * P], hp[:], Act.Lrelu,
                            bias=bl_t[:, li:li + 1], scale=1.0,
                            alpha=NEG_SLOPE)
                    else:
                        pre = prep_.tile([P, P], F32, tag="p")
                        nc.scalar.activation(pre[:], hp[:], Act.Lrelu,
                                             bias=bl_t[:, li:li + 1],
                                             scale=1.0, alpha=NEG_SLOPE)
                        nc.vector.tensor_scalar(
                            out=hT_new[:, b * P:(b + 1) * P], in0=pre[:],
                            scalar1=gs_t[:, li:li + 1],
                            scalar2=bt_t[:, li:li + 1],
                            op0=AluOp.mult, op1=AluOp.add)
                    tp = ps_tr.tile([P, P], BF16, tag="t")
                    nc.tensor.transpose(out=tp[:],
                                        in_=hT_new[:, b * P:(b + 1) * P],
                                        identity=ident_t[:])
                    stg = stgp.tile([P, P], BF16, tag="st")
                    if BN_FOLDED:
                        # PSUM->SBUF copy on the Scalar engine: keeps the
                        # in-order DVE queue free for the next block's mean
                        # multiply (head-of-line cut).
                        nc.scalar.activation(stg[:], tp[:], Act.Identity,
                                             bias=bt_t[:, 0:1], scale=1.0)
                    else:
                        nc.vector.tensor_copy(stg[:], tp[:])
                    if li < NL - 1:
                        nc.sync.dma_start(slices[li][b * P:(b + 1) * P, :],
                                          stg[:])
                    else:
                        gb = gbp.tile([P, NG], BF16, tag="gb")
                        nc.vector.tensor_scalar(
                            out=gb[:], in0=giota_t[:],
                            scalar1=gid_t[:, b:b + 1], scalar2=None,
                            op0=AluOp.is_equal)
                        half = pool_ps[:, :NG] if b < POOL_SPLIT \
                            else pool_ps[:, NG:]
                        nc.tensor.matmul(
                            out=half, lhsT=stg[:], rhs=gb[:],
                            start=(b in (0, POOL_SPLIT)),
                            stop=(b in (POOL_SPLIT - 1, NB - 1)))
                        if b == POOL_SPLIT - 1:
                            poolTa = small.tile([P, NG], F32, tag="poolTa")
                            nc.vector.tensor_copy(poolTa[:], pool_ps[:, :NG])
                            nc.sync.dma_start(payload_a[:], poolTa[:])
                            nc.gpsimd.collective_compute(
                                "AllReduce", AluOp.add, replica_groups=RG,
                                ins=[payload_a[:]], outs=[payload_all_a[:]],
                            )
                if li < NL - 1:
                    nc.gpsimd.collective_compute(
                        "AllGather", AluOp.bypass, replica_groups=RG,
                        ins=[slices[li][:]], outs=[tables[li][:]],
                    )
                hT_prev, hT_new = hT_new, hT_prev
                nc.leave_named_scope(f"layer{li}", _sc[0], False)

            # ---- epilogue ------------------------------------------------
            _sc = nc.enter_named_scope("epilogue", False)
            poolT = small.tile([P, NG], F32, tag="poolT")
            nc.vector.tensor_copy(poolT[:], pool_ps[:, NG:])
            nc.sync.dma_start(payload_b[:], poolT[:])
            nc.gpsimd.collective_compute(
                "AllReduce", AluOp.add, replica_groups=RG,
                ins=[payload_b[:]], outs=[payload_all_b[:]],
            )
            pool_acc = small.tile([P, NG], F32, tag="pacc")
            nc.sync.dma_start(pool_acc[:], payload_all_a[:])
            pool_accb = small.tile([P, NG], F32, tag="paccb")
            nc.sync.dma_start(pool_accb[:], payload_all_b[:])
            nc.vector.tensor_tensor(out=pool_acc[:], in0=pool_acc[:],
                                    in1=pool_accb[:], op=AluOp.add)
            nc.vector.tensor_tensor(out=pool_acc[:], in0=pool_acc[:],
                                    in1=ginv_bc[:], op=AluOp.mult)

            ug_ps = ps_dense.tile([P, NG], F32, tag="d")
            nc.tensor.matmul(out=ug_ps[:], lhsT=wg_t[:], rhs=uT_t[:],
                             start=True, stop=True)
            ugT = small.tile([P, NG], F32, tag="ugT")
            nc.scalar.activation(ugT[:], ug_ps[:], Act.Lrelu, bias=bg_t[:],
                                 scale=1.0, alpha=NEG_SLOPE)

            hid_ps = ps_dense.tile([P, NG], F32, tag="d")
            nc.tensor.matmul(out=hid_ps[:], lhsT=wf1a_t[:],
                             rhs=pool_acc[:], start=True, stop=False)
            nc.tensor.matmul(out=hid_ps[:], lhsT=wf1b_t[:], rhs=ugT[:],
                             start=False, stop=True)
            hidT = small.tile([P, NG], F32, tag="hidT")
            nc.scalar.activation(hidT[:], hid_ps[:], Act.Lrelu, bias=bf1_t[:],
                                 scale=1.0, alpha=NEG_SLOPE)

            y_ps = ps_dense.tile([2, NG], F32, tag="d")
            nc.tensor.matmul(out=y_ps[:], lhsT=wf2_t[:], rhs=hidT[:],
                             start=True, stop=True)
            yT = small.tile([2, NG], F32, tag="yT")
            nc.vector.tensor_scalar(out=yT[:], in0=y_ps[:], scalar1=bf2_t[:],
                                    scalar2=None, op0=AluOp.add)
            nc.sync.dma_start(y_out[:], yT[:])
            nc.leave_named_scope("epilogue", _sc[0], False)

    nc.finalize()
    _legalize_sync_waits(nc)

    common = dict(
        giota=giota_np, ident=ident_np,
        wl=W_l_eff.transpose(1, 0, 2).reshape(H, NL * H).astype(BF),
        wr=W_r_eff.transpose(1, 0, 2).reshape(H, NL * H).astype(BF),
        bl=b_l_eff.T.copy(),
        gs=gscale.T.copy(), bt=beta_np.T.copy(),
        ginv=ginv.reshape(1, NG),
        uT=u.T.copy(),
        wg=np.asarray(W_g, np.float32),
        bg=np.asarray(b_g, np.float32).reshape(P, 1),
        wf1=np.asarray(W_f1, np.float32),
        bf1=np.asarray(b_f1, np.float32).reshape(P, 1),
        wf2=np.asarray(W_f2, np.float32),
        bf2=np.asarray(b_f2, np.float32).reshape(2, 1),
    )
    in_maps = []
    for c in range(NCORES):
        cd = core_data[c]
        in_maps.append(dict(
            common,
            idx_lo=cd["idx_lo"], idx_hi=cd["idx_hi"], sp=cd["sp"],
            pg_lo=cd["pg_lo"], pg_hi=cd["pg_hi"],
            hT0=h0_pad[c * SLICE:(c + 1) * SLICE].T.astype(BF).copy(),
            gid=gids[c],
            invdeg=inv_deg[c * SLICE:(c + 1) * SLICE]
                .reshape(1, SLICE).astype(BF),
        ))

    res = run_bass_kernel_spmd(nc, in_maps, core_ids=list(range(NCORES)),
                               trace=TRACE)
    global LAST_RESULT
    LAST_RESULT = res
    return np.asarray(res.results[0]["y"]).T.astype(np.float32).copy()


TRACE = False
LAST_RESULT = None
